# revision 25
# baseline (speedup 1.0000x reference)
"""Causal self-attention with RoPE on 8 Trainium2 NeuronCores.

Sharding: Megatron-style head parallelism. 16 heads / 8 cores = 2 heads per
core. Each core computes q/k/v projections for its 2 heads (column-parallel),
full causal attention for those heads, and a partial output projection
(row-parallel slice of w_o). The host sums the 8 partial outputs.

v7: batch-pipelined emission. Measured facts this build exploits:
- bf16 matmuls stream 2 rows/cycle; fp32r 1 row/cycle -> everything on the
  PE is bf16.
- Matmuls accumulating back-to-back into the same PSUM bank stall ~143ns;
  interleaving unrelated work between them hides the commit entirely.
- The PE drops to 1.2GHz after any idle gap and needs ~3us of continuous
  work to re-ramp, so the emission interleaves batch b's attention tiles
  with batch b+1's projection chains: the PE always has independent queued
  work while ACT computes exp / DVE computes RoPE, keeping the clock high.
- exp runs on raw logits (|logit| < ~8), the causal triangle is zeroed by
  gpsimd.affine_select (idle Pool engine), diagonal tiles are narrowed to
  the valid column range, softmax denominators accumulate per-tile in PSUM
  (narrowed ones-matmuls), and normalization+output-projection are
  deferred one group and threaded into later streams.
- x travels as bf16; cos/sin DMAs precede the later x tiles (DMA queues
  are in-order; head-of-line blocking once cost 28us of startup).
"""

import numpy as np

B, T, D = 4, 2048, 2048
H, DH = 16, 128
NCORES = 8
HPC = H // NCORES  # heads per core
THETA = 10000.0

TT = 512  # projection t-tile (moving dim of q/k projection matmuls)
TQ = 512  # attention q-group width
TK = 128  # kv tile (contraction chunk of PV / partition dim of ST)


def _rope_tables(seq_len, d_head, theta):
    inv_freq = 1.0 / (theta ** (np.arange(0, d_head, 2, dtype=np.float32) / d_head))
    t = np.arange(seq_len, dtype=np.float32)
    freqs = np.einsum("i,j->ij", t, inv_freq)
    emb = np.concatenate([freqs, freqs], axis=-1)  # [T, dh]
    cosT = np.ascontiguousarray(np.cos(emb).astype(np.float32).T)  # [dh, T]
    sinT = np.ascontiguousarray(np.sin(emb).astype(np.float32).T)
    sgn = np.ones((d_head, 1), np.float32)
    sgn[: d_head // 2] = -1.0
    # halves swapped: the rotate-half multiplies then read both operands at
    # matching partition offsets
    return cosT, np.roll(sinT * sgn, d_head // 2, axis=0)


def _legalize_waits(nc, mybir):
    """Walrus refuses more than one embedded sync wait per engine
    instruction; hoist extras into standalone EventSemaphore instrs."""
    n = 0
    for f in nc.m.functions:
        for bb in f.blocks:
            out = []
            for inst in bb.instructions:
                si = inst.sync_info
                if (si and si.on_wait and len(si.on_wait) > 1
                        and not isinstance(inst, mybir.InstEventSemaphore)):
                    for w in si.on_wait[:-1]:
                        out.append(mybir.InstEventSemaphore(
                            name=f"WH-{n}", engine=inst.engine,
                            sync_info=mybir.SyncInfo(
                                on_wait=[w], on_update=[])))
                        n += 1
                    inst.sync_info = mybir.SyncInfo(
                        on_wait=[si.on_wait[-1]],
                        on_update=list(si.on_update))
                out.append(inst)
            bb.instructions = out
    return n


def _build_nc(b_sz, t_sz, d_sz, legalize=True):
    import concourse.bass as bass
    import concourse.tile as tile
    from concourse import mybir

    f32 = mybir.dt.float32
    f32r = mybir.dt.float32r
    bf16 = mybir.dt.bfloat16
    EXP = mybir.ActivationFunctionType.Exp

    DC = d_sz // 128         # contraction chunks
    NQG = t_sz // TQ         # q groups per (batch, head)
    NKT = t_sz // TK         # kv tiles
    KPG = TQ // TK           # kv tiles per q group (diagonal span)

    nc = bass.Bass("TRN2", target_bir_lowering=False, debug=False,
                   enable_asserts=False, dynamic_dma_scratch_size=2048)

    xT = nc.dram_tensor("xT", [b_sz, d_sz, t_sz], bf16, kind="ExternalInput")
    wq = nc.dram_tensor("wq", [d_sz, HPC * DH], bf16, kind="ExternalInput")
    wk = nc.dram_tensor("wk", [d_sz, HPC * DH], bf16, kind="ExternalInput")
    wv = nc.dram_tensor("wv", [d_sz, HPC * DH], bf16, kind="ExternalInput")
    wo = nc.dram_tensor("wo", [HPC * DH, d_sz], bf16, kind="ExternalInput")
    cos = nc.dram_tensor("cos", [DH, t_sz], f32, kind="ExternalInput")
    sin = nc.dram_tensor("sin", [DH, t_sz], f32, kind="ExternalInput")
    one = nc.dram_tensor("one", [128, 128], f32, kind="ExternalInput")
    oneb = nc.dram_tensor("oneb", [128, 128], bf16, kind="ExternalInput")
    y = nc.dram_tensor("y", [b_sz, t_sz, d_sz], f32, kind="ExternalOutput")

    xT_r = xT.ap().rearrange("b (dc p) t -> b p dc t", p=128)
    wq_r = wq.ap().rearrange("(dc p) n -> p dc n", p=128)
    wk_r = wk.ap().rearrange("(dc p) n -> p dc n", p=128)
    wv_r = wv.ap().rearrange("(dc p) n -> p dc n", p=128)
    wo_r = wo.ap().rearrange("(h p) n -> p h n", p=128)
    y_r = y.ap()

    dg_off = {0: 0, 1: 128, 2: 256, 3: 256}

    with tile.TileContext(nc) as tc:
        with (
            tc.tile_pool(name="consts", bufs=1) as consts,
            tc.tile_pool(name="wpool", bufs=1) as wpool,
            tc.tile_pool(name="qkv", bufs=2) as qkv,
            tc.tile_pool(name="xpool", bufs=4) as xpool,
            tc.tile_pool(name="rope", bufs=2) as rope,
            tc.tile_pool(name="pex", bufs=6) as pexp,
            tc.tile_pool(name="sax", bufs=1) as sax,
            tc.tile_pool(name="otn", bufs=6) as otnp,
            tc.tile_pool(name="psS", bufs=2, space="PSUM") as psS,
            tc.tile_pool(name="psO", bufs=2, space="PSUM") as psO,
            tc.tile_pool(name="psP", bufs=1, space="PSUM") as psP,
            tc.tile_pool(name="psR", bufs=1, space="PSUM") as psR,
            tc.tile_pool(name="psY", bufs=2, space="PSUM") as psY,
        ):
            cos_sb = consts.tile([DH, t_sz], f32)
            sin_sb = consts.tile([DH, t_sz], f32)
            onesb_sb = consts.tile([128, 1], bf16)
            onesrow_sb = consts.tile([1, 128], f32r)

            wq_sb = wpool.tile([128, DC, HPC * DH], bf16)
            wk_sb = wpool.tile([128, DC, HPC * DH], bf16)
            wv_sb = wpool.tile([128, DC, HPC * DH], bf16)
            wo_sb = wpool.tile([128, HPC, d_sz], bf16)

            xt_first = xpool.tile([128, DC, TT], bf16, tag="xt",
                                  name="xt_first")
            for dc in range(DC):
                nc.sync.dma_start(xt_first[:, dc, :],
                                  xT_r[0, :, dc, 0:TT])
                nc.sync.dma_start(wq_sb[:, dc, :], wq_r[:, dc, :])
                nc.sync.dma_start(wk_sb[:, dc, :], wk_r[:, dc, :])
                nc.sync.dma_start(wv_sb[:, dc, :], wv_r[:, dc, :])
            for i in range(t_sz // TT):
                sl = slice(i * TT, (i + 1) * TT)
                nc.sync.dma_start(cos_sb[:, sl], cos.ap()[:, sl])
                nc.sync.dma_start(sin_sb[:, sl], sin.ap()[:, sl])

            def load_consts():
                nc.sync.dma_start(onesb_sb[:], oneb.ap()[:, 0:1])
                nc.sync.dma_start(onesrow_sb[:],
                                  one.ap()[0:1, :].bitcast(f32r))
                for hh in range(HPC):
                    for nch in range(d_sz // 512):
                        nsl = slice(nch * 512, (nch + 1) * 512)
                        nc.sync.dma_start(wo_sb[:, hh, nsl],
                                          wo_r[:, hh, nsl])

            pending_norm = []
            pending_y = []

            def pop_norm():
                if pending_norm:
                    pending_norm.pop(0)()

            def pop_y(k=2):
                for _ in range(k):
                    if pending_y:
                        pending_y.pop(0)()

            def prefetch_x(b):
                tiles = []
                for tt in range(t_sz // TT):
                    if b == 0 and tt == 0:
                        tiles.append(xt_first)
                        continue
                    xt = xpool.tile([128, DC, TT], bf16, tag="xt", name="xt")
                    tsl = slice(tt * TT, (tt + 1) * TT)
                    for dc in range(DC):
                        nc.sync.dma_start(xt[:, dc, :], xT_r[b, :, dc, tsl])
                    tiles.append(xt)
                return tiles

            def gen_proj(b, xts, sets):
                """Projections + RoPE for batch b. Yields after each chain
                so the driver can interleave attention work."""
                qT = [qkv.tile([DH, t_sz], bf16, tag=f"qT{h}",
                               name=f"qT{h}") for h in range(HPC)]
                kT = [qkv.tile([DH, t_sz], bf16, tag=f"kT{h}",
                               name=f"kT{h}") for h in range(HPC)]
                vv = qkv.tile([128, NKT, HPC * DH], bf16, tag="v", name="v")
                sets[b] = (qT, kT, vv)

                def rope(dst, pp, tsl):
                    sh = rope_pool.tile([DH, TT], bf16, tag="sh")
                    nc.vector.tensor_mul(
                        sh[0:64, :], pp[64:128, :], sin_sb[64:128, tsl])
                    nc.vector.tensor_mul(
                        sh[64:128, :], pp[0:64, :], sin_sb[0:64, tsl])
                    t2 = rope_pool.tile([DH, TT], bf16, tag="t2")
                    nc.vector.tensor_mul(t2[:], pp[:], cos_sb[:, tsl])
                    nc.vector.tensor_add(dst[:, tsl], t2[:], sh[:])

                for tt in range(t_sz // TT):
                    tsl = slice(tt * TT, (tt + 1) * TT)
                    xt = xts[tt]
                    if b == 0 and tt == 0:
                        load_consts()
                    for h in range(HPC):
                        hs = slice(h * DH, (h + 1) * DH)
                        for dst, w_sb in ((qT[h], wq_sb), (kT[h], wk_sb)):
                            pp = psP.tile([DH, TT], f32, tag="pj")
                            for dc in range(DC):
                                nc.tensor.matmul(
                                    pp[:], w_sb[:, dc, hs], xt[:, dc, :],
                                    start=(dc == 0), stop=(dc == DC - 1),
                                )
                            rope(dst, pp, tsl)
                            yield
                    for ts2 in range(TT // TK):
                        vp = psP.tile([TK, HPC * DH], f32, tag="pj")
                        for dc in range(DC):
                            nc.tensor.matmul(
                                vp[:],
                                xt[:, dc, ts2 * TK:(ts2 + 1) * TK],
                                wv_sb[:, dc, :],
                                start=(dc == 0), stop=(dc == DC - 1),
                            )
                        kv_i = tt * (TT // TK) + ts2
                        nc.scalar.copy(vv[:, kv_i, :], vp[:])
                        yield

            rope_pool = rope

            def gen_attn(b, sets):
                """Attention for batch b. Yields after each kv tile."""
                qT, kT, vv = sets.pop(b)
                otn_tiles = {}
                for h in range(HPC):
                    for qi in range(NQG):
                        nkv = KPG * (qi + 1)
                        outp = psO.tile([DH, TQ], f32, tag="outT")
                        sump = psR.tile([1, TQ], f32, tag="nrm",
                                        name="sump")
                        tiles = []
                        for ki in range(nkv):
                            dg = ki - KPG * qi
                            off = dg_off[dg] if dg >= 0 else 0
                            qsl = slice(qi * TQ + off, (qi + 1) * TQ)
                            stp = psS.tile([TK, TQ], f32, tag="st")
                            nc.tensor.matmul(
                                stp[:, off:],
                                kT[h][:, ki * TK:(ki + 1) * TK],
                                qT[h][:, qsl],
                                start=True, stop=True,
                            )
                            pex = pexp.tile([TK, TQ], bf16, tag="pex")
                            nc.scalar.activation(pex[:, off:], stp[:, off:],
                                                 EXP)
                            if dg >= 0:
                                base = off - (dg * TK)
                                blk = 2 * TK if dg == 3 else TK
                                nc.gpsimd.affine_select(
                                    out=pex[:, off:off + blk],
                                    in_=pex[:, off:off + blk],
                                    compare_op=mybir.AluOpType.is_ge,
                                    fill=0.0,
                                    base=base,
                                    pattern=[[1, blk]],
                                    channel_multiplier=-1,
                                )
                            tiles.append((ki, off, pex))
                            if len(tiles) > 1:
                                pki, poff, ppex = tiles.pop(0)
                                nc.tensor.matmul(
                                    outp[:, poff:],
                                    vv[:, pki, h * DH:(h + 1) * DH],
                                    ppex[:, poff:],
                                    start=(pki == 0), stop=False,
                                    skip_group_check=True,
                                )
                                nc.tensor.matmul(
                                    sump[:, poff:],
                                    onesb_sb[:],
                                    ppex[:, poff:],
                                    start=(pki == 0), stop=False,
                                    skip_group_check=True,
                                )
                                pop_y(2)
                            yield
                        pki, poff, ppex = tiles.pop(0)
                        nc.tensor.matmul(
                            outp[:, poff:],
                            vv[:, pki, h * DH:(h + 1) * DH],
                            ppex[:, poff:],
                            start=(pki == 0), stop=True,
                            skip_group_check=True,
                        )
                        nc.tensor.matmul(
                            sump[:, poff:],
                            onesb_sb[:],
                            ppex[:, poff:],
                            start=(pki == 0), stop=True,
                            skip_group_check=True,
                        )
                        pop_y(2)

                        def norm(h=h, qi=qi, outp=outp, sump=sump, b=b,
                                 ot=otn_tiles):
                            ssb = sax.tile([1, TQ], f32r, tag="ssb", bufs=2,
                                           name="ssb")
                            nc.scalar.copy(ssb[:], sump[:])
                            # rbc rides the psY rotation (a separate psR tag
                            # would need a 9th PSUM bank)
                            rbc = psY.tile([DH, TQ], f32, tag="y",
                                           name="rbc")
                            nc.tensor.matmul(rbc[:], onesrow_sb[:], ssb[:],
                                             start=True, stop=True)
                            rcp = sax.tile([DH, TQ], f32, tag="rcp", bufs=2,
                                           name="rcp")
                            nc.vector.reciprocal(rcp[:], rbc[:])
                            otn = otnp.tile([DH, TQ], bf16, tag="otn",
                                            name="otn")
                            nc.vector.tensor_mul(otn[:], outp[:], rcp[:])
                            ot[(h, qi)] = otn
                            if h != HPC - 1:
                                return

                            def make_ytile(tc2, nch, qi=qi, b=b, ot=ot):
                                def emit():
                                    tq0 = qi * TQ + tc2 * TK
                                    yp = psY.tile([TK, 512], f32,
                                                  tag="y", name="yp")
                                    for hh in range(HPC):
                                        nc.tensor.matmul(
                                            yp[:],
                                            ot[(hh, qi)][
                                                :, tc2 * TK:(tc2 + 1) * TK],
                                            wo_sb[:, hh,
                                                  nch * 512:(nch + 1) * 512],
                                            start=(hh == 0),
                                            stop=(hh == HPC - 1),
                                        )
                                    ysb = pexp.tile([TK, 512], f32,
                                                    tag="ysb", bufs=3,
                                                    name="ysb")
                                    if nch % 2 == 0:
                                        nc.scalar.copy(ysb[:], yp[:])
                                    else:
                                        nc.vector.tensor_copy(ysb[:], yp[:])
                                    nc.sync.dma_start(
                                        y_r[b, tq0:tq0 + TK,
                                            nch * 512:(nch + 1) * 512],
                                        ysb[:])
                                return emit

                            for tc2 in range(TQ // TK):
                                for nch in range(d_sz // 512):
                                    pending_y.append(make_ytile(tc2, nch))

                        pending_norm.append(norm)
                        if len(pending_norm) > 1:
                            pending_norm.pop(0)()
                pop_norm()

            # ---- driver: attention of batch b interleaved with the
            # projections of batch b+1 (the PE always has queued work) ----
            sets = {}
            xts = prefetch_x(0)
            ga = gen_proj(0, xts, sets)
            for _ in ga:
                pass
            for b in range(b_sz):
                gb = gen_attn(b, sets)
                if b + 1 < b_sz:
                    xts = prefetch_x(b + 1)
                    ga = gen_proj(b + 1, xts, sets)
                    na, nb = 32, 80  # quanta counts for pacing
                    adone = bdone = 0
                    alive_a = alive_b = True
                    while alive_b or alive_a:
                        if alive_b:
                            try:
                                next(gb)
                                bdone += 1
                            except StopIteration:
                                alive_b = False
                        if alive_a and (not alive_b
                                        or adone * nb <= bdone * na):
                            try:
                                next(ga)
                                adone += 1
                            except StopIteration:
                                alive_a = False
                else:
                    for _ in gb:
                        pass
            for fn in pending_norm:
                fn()
            pop_y(len(pending_y))
    if legalize:
        _legalize_waits(nc, mybir)
    return nc


_NC_CACHE = {}
LAST_RESULT = None


def _get_nc(b_sz, t_sz, d_sz):
    key = (b_sz, t_sz, d_sz)
    if key not in _NC_CACHE:
        _NC_CACHE[key] = _build_nc(b_sz, t_sz, d_sz)
    return _NC_CACHE[key]


def kernel(x, w_q, w_k, w_v, w_o):
    import ml_dtypes
    from concourse.bass_utils import run_bass_kernel_spmd

    bf = ml_dtypes.bfloat16
    b_sz, t_sz, d_sz = x.shape
    scale = np.float32(1.0 / np.sqrt(DH))

    xT = np.ascontiguousarray(
        np.asarray(x, np.float32).transpose(0, 2, 1)).astype(bf)
    w_q = np.asarray(w_q, np.float32)
    w_k = np.asarray(w_k, np.float32)
    w_v = np.asarray(w_v, np.float32)
    w_o = np.asarray(w_o, np.float32)
    cosT, sinT = _rope_tables(t_sz, DH, THETA)

    in_maps = []
    for c in range(NCORES):
        cs = slice(c * HPC * DH, (c + 1) * HPC * DH)
        in_maps.append({
            "xT": xT,
            "wq": np.ascontiguousarray(w_q[:, cs] * scale).astype(bf),
            "wk": np.ascontiguousarray(w_k[:, cs]).astype(bf),
            "wv": np.ascontiguousarray(w_v[:, cs]).astype(bf),
            "wo": np.ascontiguousarray(w_o[cs, :]).astype(bf),
            "cos": cosT,
            "sin": sinT,
            "one": np.ones((128, 128), np.float32),
            "oneb": np.ones((128, 128), bf),
        })

    nc = _get_nc(b_sz, t_sz, d_sz)
    res = run_bass_kernel_spmd(nc, in_maps, core_ids=list(range(NCORES)))
    global LAST_RESULT
    LAST_RESULT = res

    out = res.results[0]["y"].astype(np.float32, copy=True)
    for c in range(1, NCORES):
        out += res.results[c]["y"]
    return out


# revision 28
# speedup vs baseline: 1.1740x; 1.1740x over previous
"""Causal self-attention with RoPE on 8 Trainium2 NeuronCores.

Sharding: Megatron-style head parallelism. 16 heads / 8 cores = 2 heads per
core. Each core computes q/k/v projections for its 2 heads (column-parallel),
full causal attention for those heads, and a partial output projection
(row-parallel slice of w_o). The host sums the 8 partial outputs.

v7: batch-pipelined emission. Measured facts this build exploits:
- bf16 matmuls stream 2 rows/cycle; fp32r 1 row/cycle -> everything on the
  PE is bf16.
- Matmuls accumulating back-to-back into the same PSUM bank stall ~143ns;
  interleaving unrelated work between them hides the commit entirely.
- The PE drops to 1.2GHz after any idle gap and needs ~3us of continuous
  work to re-ramp, so the emission interleaves batch b's attention tiles
  with batch b+1's projection chains: the PE always has independent queued
  work while ACT computes exp / DVE computes RoPE, keeping the clock high.
- exp runs on raw logits (|logit| < ~8), the causal triangle is zeroed by
  gpsimd.affine_select (idle Pool engine), diagonal tiles are narrowed to
  the valid column range, softmax denominators accumulate per-tile in PSUM
  (narrowed ones-matmuls), and normalization+output-projection are
  deferred one group and threaded into later streams.
- x travels as bf16; cos/sin DMAs precede the later x tiles (DMA queues
  are in-order; head-of-line blocking once cost 28us of startup).
"""

import numpy as np

B, T, D = 4, 2048, 2048
H, DH = 16, 128
NCORES = 8
HPC = H // NCORES  # heads per core
THETA = 10000.0

TT = 512  # projection t-tile (moving dim of q/k projection matmuls)
TQ = 512  # attention q-group width
TK = 128  # kv tile (contraction chunk of PV / partition dim of ST)


def _rope_tables(seq_len, d_head, theta):
    inv_freq = 1.0 / (theta ** (np.arange(0, d_head, 2, dtype=np.float32) / d_head))
    t = np.arange(seq_len, dtype=np.float32)
    freqs = np.einsum("i,j->ij", t, inv_freq)
    emb = np.concatenate([freqs, freqs], axis=-1)  # [T, dh]
    cosT = np.ascontiguousarray(np.cos(emb).astype(np.float32).T)  # [dh, T]
    sinT = np.ascontiguousarray(np.sin(emb).astype(np.float32).T)
    sgn = np.ones((d_head, 1), np.float32)
    sgn[: d_head // 2] = -1.0
    # halves swapped: the rotate-half multiplies then read both operands at
    # matching partition offsets
    return cosT, np.roll(sinT * sgn, d_head // 2, axis=0)


def _legalize_waits(nc, mybir):
    """Walrus refuses more than one embedded sync wait per engine
    instruction; hoist extras into standalone EventSemaphore instrs."""
    n = 0
    for f in nc.m.functions:
        for bb in f.blocks:
            out = []
            for inst in bb.instructions:
                si = inst.sync_info
                if (si and si.on_wait and len(si.on_wait) > 1
                        and not isinstance(inst, mybir.InstEventSemaphore)):
                    for w in si.on_wait[:-1]:
                        out.append(mybir.InstEventSemaphore(
                            name=f"WH-{n}", engine=inst.engine,
                            sync_info=mybir.SyncInfo(
                                on_wait=[w], on_update=[])))
                        n += 1
                    inst.sync_info = mybir.SyncInfo(
                        on_wait=[si.on_wait[-1]],
                        on_update=list(si.on_update))
                out.append(inst)
            bb.instructions = out
    return n


def _build_nc(b_sz, t_sz, d_sz, legalize=True):
    import concourse.bass as bass
    import concourse.tile as tile
    from concourse import mybir

    f32 = mybir.dt.float32
    f32r = mybir.dt.float32r
    bf16 = mybir.dt.bfloat16
    EXP = mybir.ActivationFunctionType.Exp

    DC = d_sz // 128         # contraction chunks
    NQG = t_sz // TQ         # q groups per (batch, head)
    NKT = t_sz // TK         # kv tiles
    KPG = TQ // TK           # kv tiles per q group (diagonal span)

    nc = bass.Bass("TRN2", target_bir_lowering=False, debug=False,
                   enable_asserts=False, dynamic_dma_scratch_size=2048)

    xT = nc.dram_tensor("xT", [b_sz, d_sz, t_sz], bf16, kind="ExternalInput")
    wq = nc.dram_tensor("wq", [d_sz, HPC * DH], bf16, kind="ExternalInput")
    wk = nc.dram_tensor("wk", [d_sz, HPC * DH], bf16, kind="ExternalInput")
    wv = nc.dram_tensor("wv", [d_sz, HPC * DH], bf16, kind="ExternalInput")
    wo = nc.dram_tensor("wo", [HPC * DH, d_sz], bf16, kind="ExternalInput")
    cos = nc.dram_tensor("cos", [DH, t_sz], f32, kind="ExternalInput")
    sin = nc.dram_tensor("sin", [DH, t_sz], f32, kind="ExternalInput")
    one = nc.dram_tensor("one", [128, 128], f32, kind="ExternalInput")
    oneb = nc.dram_tensor("oneb", [128, 128], bf16, kind="ExternalInput")
    y = nc.dram_tensor("y", [b_sz, t_sz, d_sz], f32, kind="ExternalOutput")

    xT_r = xT.ap().rearrange("b (dc p) t -> b p dc t", p=128)
    wq_r = wq.ap().rearrange("(dc p) n -> p dc n", p=128)
    wk_r = wk.ap().rearrange("(dc p) n -> p dc n", p=128)
    wv_r = wv.ap().rearrange("(dc p) n -> p dc n", p=128)
    wo_r = wo.ap().rearrange("(h p) n -> p h n", p=128)
    y_r = y.ap()

    dg_off = {0: 0, 1: 128, 2: 256, 3: 256}

    with tile.TileContext(nc) as tc:
        with (
            tc.tile_pool(name="consts", bufs=1) as consts,
            tc.tile_pool(name="wpool", bufs=1) as wpool,
            tc.tile_pool(name="qkv", bufs=2) as qkv,
            tc.tile_pool(name="xpool", bufs=4) as xpool,
            tc.tile_pool(name="rope", bufs=2) as rope,
            tc.tile_pool(name="pex", bufs=6) as pexp,
            tc.tile_pool(name="sax", bufs=1) as sax,
            tc.tile_pool(name="otn", bufs=6) as otnp,
            tc.tile_pool(name="psS", bufs=2, space="PSUM") as psS,
            tc.tile_pool(name="psO", bufs=2, space="PSUM") as psO,
            tc.tile_pool(name="psP", bufs=1, space="PSUM") as psP,
            tc.tile_pool(name="psR", bufs=1, space="PSUM") as psR,
            tc.tile_pool(name="psY", bufs=1, space="PSUM") as psY,
        ):
            cos_sb = consts.tile([DH, t_sz], f32)
            sin_sb = consts.tile([DH, t_sz], f32)
            onesb_sb = consts.tile([128, 1], bf16)
            onesrow_sb = consts.tile([1, 128], f32r)

            wq_sb = wpool.tile([128, DC, HPC * DH], bf16)
            wk_sb = wpool.tile([128, DC, HPC * DH], bf16)
            wv_sb = wpool.tile([128, DC, HPC * DH], bf16)
            wo_sb = wpool.tile([128, HPC, d_sz], bf16)

            xt_first = xpool.tile([128, DC, TT], bf16, tag="xt",
                                  name="xt_first")
            for dc in range(DC):
                nc.sync.dma_start(xt_first[:, dc, :],
                                  xT_r[0, :, dc, 0:TT])
                nc.sync.dma_start(wq_sb[:, dc, :], wq_r[:, dc, :])
                nc.sync.dma_start(wk_sb[:, dc, :], wk_r[:, dc, :])
                nc.sync.dma_start(wv_sb[:, dc, :], wv_r[:, dc, :])
            for i in range(t_sz // TT):
                sl = slice(i * TT, (i + 1) * TT)
                nc.sync.dma_start(cos_sb[:, sl], cos.ap()[:, sl])
                nc.sync.dma_start(sin_sb[:, sl], sin.ap()[:, sl])

            def load_consts():
                nc.sync.dma_start(onesb_sb[:], oneb.ap()[:, 0:1])
                nc.sync.dma_start(onesrow_sb[:],
                                  one.ap()[0:1, :].bitcast(f32r))
                for hh in range(HPC):
                    for nch in range(d_sz // 512):
                        nsl = slice(nch * 512, (nch + 1) * 512)
                        nc.sync.dma_start(wo_sb[:, hh, nsl],
                                          wo_r[:, hh, nsl])

            pending_norm = []
            pending_y = []

            def pop_norm():
                if pending_norm:
                    pending_norm.pop(0)()

            def pop_y(k=2):
                for _ in range(k):
                    if pending_y:
                        pending_y.pop(0)()

            def prefetch_x(b):
                tiles = []
                for tt in range(t_sz // TT):
                    if b == 0 and tt == 0:
                        tiles.append(xt_first)
                        continue
                    xt = xpool.tile([128, DC, TT], bf16, tag="xt", name="xt")
                    tsl = slice(tt * TT, (tt + 1) * TT)
                    for dc in range(DC):
                        nc.sync.dma_start(xt[:, dc, :], xT_r[b, :, dc, tsl])
                    tiles.append(xt)
                return tiles

            def gen_proj(b, xts, sets):
                """Projections + RoPE for batch b. Yields after each chain
                so the driver can interleave attention work."""
                qT = [qkv.tile([DH, t_sz], bf16, tag=f"qT{h}",
                               name=f"qT{h}") for h in range(HPC)]
                kT = [qkv.tile([DH, t_sz], bf16, tag=f"kT{h}",
                               name=f"kT{h}") for h in range(HPC)]
                vv = qkv.tile([128, NKT, HPC * DH], bf16, tag="v", name="v")
                sets[b] = (qT, kT, vv)

                def rope(dst, pp, tsl):
                    sh = rope_pool.tile([DH, TT], bf16, tag="sh")
                    nc.vector.tensor_mul(
                        sh[0:64, :], pp[64:128, :], sin_sb[64:128, tsl])
                    nc.vector.tensor_mul(
                        sh[64:128, :], pp[0:64, :], sin_sb[0:64, tsl])
                    t2 = rope_pool.tile([DH, TT], bf16, tag="t2")
                    nc.vector.tensor_mul(t2[:], pp[:], cos_sb[:, tsl])
                    nc.vector.tensor_add(dst[:, tsl], t2[:], sh[:])

                for tt in range(t_sz // TT):
                    tsl = slice(tt * TT, (tt + 1) * TT)
                    xt = xts[tt]
                    if b == 0 and tt == 0:
                        load_consts()
                    for h in range(HPC):
                        hs = slice(h * DH, (h + 1) * DH)
                        for dst, w_sb in ((qT[h], wq_sb), (kT[h], wk_sb)):
                            pp = psP.tile([DH, TT], f32, tag="pj")
                            for dc in range(DC):
                                nc.tensor.matmul(
                                    pp[:], w_sb[:, dc, hs], xt[:, dc, :],
                                    start=(dc == 0), stop=(dc == DC - 1),
                                )
                            rope(dst, pp, tsl)
                            pop_y(1)
                            yield
                    for ts2 in range(TT // TK):
                        vp = psP.tile([TK, HPC * DH], f32, tag="pj")
                        for dc in range(DC):
                            nc.tensor.matmul(
                                vp[:],
                                xt[:, dc, ts2 * TK:(ts2 + 1) * TK],
                                wv_sb[:, dc, :],
                                start=(dc == 0), stop=(dc == DC - 1),
                            )
                        kv_i = tt * (TT // TK) + ts2
                        nc.scalar.copy(vv[:, kv_i, :], vp[:])
                        pop_y(1)
                        yield

            rope_pool = rope

            def gen_attn(b, sets):
                """Attention for batch b. Yields after each kv tile."""
                qT, kT, vv = sets.pop(b)
                otn_tiles = {}
                for h in range(HPC):
                    for qi in range(NQG):
                        nkv = KPG * (qi + 1)
                        outp = psO.tile([DH, TQ], f32, tag="outT")
                        sump = psR.tile([1, TQ], f32, tag="nrm",
                                        name="sump")
                        tiles = []
                        for ki in range(nkv):
                            dg = ki - KPG * qi
                            off = dg_off[dg] if dg >= 0 else 0
                            qsl = slice(qi * TQ + off, (qi + 1) * TQ)
                            stp = psS.tile([TK, TQ], f32, tag="st")
                            nc.tensor.matmul(
                                stp[:, off:],
                                kT[h][:, ki * TK:(ki + 1) * TK],
                                qT[h][:, qsl],
                                start=True, stop=True,
                            )
                            pex = pexp.tile([TK, TQ], bf16, tag="pex")
                            nc.scalar.activation(pex[:, off:], stp[:, off:],
                                                 EXP)
                            if dg >= 0:
                                base = off - (dg * TK)
                                blk = 2 * TK if dg == 3 else TK
                                nc.gpsimd.affine_select(
                                    out=pex[:, off:off + blk],
                                    in_=pex[:, off:off + blk],
                                    compare_op=mybir.AluOpType.is_ge,
                                    fill=0.0,
                                    base=base,
                                    pattern=[[1, blk]],
                                    channel_multiplier=-1,
                                )
                            tiles.append((ki, off, pex))
                            if len(tiles) > 1:
                                pki, poff, ppex = tiles.pop(0)
                                nc.tensor.matmul(
                                    outp[:, poff:],
                                    vv[:, pki, h * DH:(h + 1) * DH],
                                    ppex[:, poff:],
                                    start=(pki == 0), stop=False,
                                    skip_group_check=True,
                                )
                                nc.tensor.matmul(
                                    sump[:, poff:],
                                    onesb_sb[:],
                                    ppex[:, poff:],
                                    start=(pki == 0), stop=False,
                                    skip_group_check=True,
                                )
                                pop_y(1)
                            yield
                        pki, poff, ppex = tiles.pop(0)
                        nc.tensor.matmul(
                            outp[:, poff:],
                            vv[:, pki, h * DH:(h + 1) * DH],
                            ppex[:, poff:],
                            start=(pki == 0), stop=True,
                            skip_group_check=True,
                        )
                        nc.tensor.matmul(
                            sump[:, poff:],
                            onesb_sb[:],
                            ppex[:, poff:],
                            start=(pki == 0), stop=True,
                            skip_group_check=True,
                        )
                        pop_y(1)

                        def norm(h=h, qi=qi, outp=outp, sump=sump, b=b,
                                 ot=otn_tiles):
                            ssb = sax.tile([1, TQ], f32r, tag="ssb", bufs=2,
                                           name="ssb")
                            nc.scalar.copy(ssb[:], sump[:])
                            rbc = psR.tile([DH, TQ], f32, tag="bc",
                                           name="rbc")
                            nc.tensor.matmul(rbc[:], onesrow_sb[:], ssb[:],
                                             start=True, stop=True)
                            rcp = sax.tile([DH, TQ], f32, tag="rcp", bufs=2,
                                           name="rcp")
                            nc.vector.reciprocal(rcp[:], rbc[:])
                            otn = otnp.tile([DH, TQ], bf16, tag="otn",
                                            name="otn")
                            nc.vector.tensor_mul(otn[:], outp[:], rcp[:])
                            ot[(h, qi)] = otn
                            if h != HPC - 1:
                                return

                            def make_ytile(tc2, nch, qi=qi, b=b, ot=ot):
                                def emit():
                                    tq0 = qi * TQ + tc2 * TK
                                    yp = psY.tile([TK, 512], f32,
                                                  tag="y", name="yp")
                                    for hh in range(HPC):
                                        nc.tensor.matmul(
                                            yp[:],
                                            ot[(hh, qi)][
                                                :, tc2 * TK:(tc2 + 1) * TK],
                                            wo_sb[:, hh,
                                                  nch * 512:(nch + 1) * 512],
                                            start=(hh == 0),
                                            stop=(hh == HPC - 1),
                                        )
                                    ysb = pexp.tile([TK, 512], f32,
                                                    tag="ysb", bufs=3,
                                                    name="ysb")
                                    if nch % 2 == 0:
                                        nc.scalar.copy(ysb[:], yp[:])
                                    else:
                                        nc.vector.tensor_copy(ysb[:], yp[:])
                                    nc.sync.dma_start(
                                        y_r[b, tq0:tq0 + TK,
                                            nch * 512:(nch + 1) * 512],
                                        ysb[:])
                                return emit

                            for tc2 in range(TQ // TK):
                                for nch in range(d_sz // 512):
                                    pending_y.append(make_ytile(tc2, nch))

                        pending_norm.append(norm)
                        if len(pending_norm) > 1:
                            pending_norm.pop(0)()
                pop_norm()

            # ---- driver: attention of batch b interleaved with the
            # projections of batch b+1 (the PE always has queued work) ----
            sets = {}
            xts = prefetch_x(0)
            ga = gen_proj(0, xts, sets)
            for _ in ga:
                pass
            for b in range(b_sz):
                gb = gen_attn(b, sets)
                if b + 1 < b_sz:
                    xts = prefetch_x(b + 1)
                    ga = gen_proj(b + 1, xts, sets)
                    na, nb = 32, 80  # quanta counts for pacing
                    adone = bdone = 0
                    alive_a = alive_b = True
                    while alive_b or alive_a:
                        if alive_b:
                            try:
                                next(gb)
                                bdone += 1
                            except StopIteration:
                                alive_b = False
                        if alive_a and (not alive_b
                                        or adone * nb <= bdone * na):
                            try:
                                next(ga)
                                adone += 1
                            except StopIteration:
                                alive_a = False
                else:
                    for _ in gb:
                        pass
            for fn in pending_norm:
                fn()
            pop_y(len(pending_y))
    if legalize:
        _legalize_waits(nc, mybir)
    return nc


_NC_CACHE = {}
LAST_RESULT = None


def _get_nc(b_sz, t_sz, d_sz):
    key = (b_sz, t_sz, d_sz)
    if key not in _NC_CACHE:
        _NC_CACHE[key] = _build_nc(b_sz, t_sz, d_sz)
    return _NC_CACHE[key]


def kernel(x, w_q, w_k, w_v, w_o):
    import ml_dtypes
    from concourse.bass_utils import run_bass_kernel_spmd

    bf = ml_dtypes.bfloat16
    b_sz, t_sz, d_sz = x.shape
    scale = np.float32(1.0 / np.sqrt(DH))

    xT = np.ascontiguousarray(
        np.asarray(x, np.float32).transpose(0, 2, 1)).astype(bf)
    w_q = np.asarray(w_q, np.float32)
    w_k = np.asarray(w_k, np.float32)
    w_v = np.asarray(w_v, np.float32)
    w_o = np.asarray(w_o, np.float32)
    cosT, sinT = _rope_tables(t_sz, DH, THETA)

    in_maps = []
    for c in range(NCORES):
        cs = slice(c * HPC * DH, (c + 1) * HPC * DH)
        in_maps.append({
            "xT": xT,
            "wq": np.ascontiguousarray(w_q[:, cs] * scale).astype(bf),
            "wk": np.ascontiguousarray(w_k[:, cs]).astype(bf),
            "wv": np.ascontiguousarray(w_v[:, cs]).astype(bf),
            "wo": np.ascontiguousarray(w_o[cs, :]).astype(bf),
            "cos": cosT,
            "sin": sinT,
            "one": np.ones((128, 128), np.float32),
            "oneb": np.ones((128, 128), bf),
        })

    nc = _get_nc(b_sz, t_sz, d_sz)
    res = run_bass_kernel_spmd(nc, in_maps, core_ids=list(range(NCORES)))
    global LAST_RESULT
    LAST_RESULT = res

    out = res.results[0]["y"].astype(np.float32, copy=True)
    for c in range(1, NCORES):
        out += res.results[c]["y"]
    return out
